# revision 1
# baseline (speedup 1.0000x reference)
"""Masked batched dot-product attention on 8 Trainium2 NeuronCores (Bass/Tile).

Reference computation (per batch b):
    scores = Q @ K^T / sqrt(D)                  [Q, K]
    scores[:, k >= valid_len[b]] = -1e6
    attn   = softmax(scores, axis=-1)
    out    = attn @ V                           [Q, V]

Strategy:
  - Data-parallel over the batch dim: 32 batches -> 8 cores x 4 slots.
    Batches are assigned to (slot, core) sorted by valid_len so all cores
    run the same (SPMD) trace while each slot's K-extent is trimmed to the
    slot-wise max number of 128-wide K chunks.
  - Per (slot, k-chunk), transposed score layout [k, q]:
      scoresT = KT_chunk.T @ QT                  (PE, bf16, PSUM f32)
      expT    = exp(scoresT/sqrt(D) + bias[k])   (ScalarE -> SBUF bf16;
                bias is -1e9 on masked k so masked weights are exactly 0;
                chunks below every core's valid_len skip the bias so the
                first exps don't wait for the bias DMA)
      O^T    += V_chunk.T-contraction of expT    (PE, accumulated in PSUM)
      acc    += expT                             (VectorE, bf16 2x mode)
  - All chunks across slots form one flat schedule with a depth-2 software
    pipeline: chunk g's AV matmuls are emitted after chunk g+2's score
    matmuls, so the in-order PE queue never stalls the next slot's scores
    behind an AV that waits on this slot's last exp.
  - The kernel lead loads the first chunk's kt/qt halves on three separate
    engine queues (SP/ACT/DVE) so their HWDGE setups pipeline; dummy
    matmuls warm the PE p-state while they land.
  - O^T leaves PSUM via DVE copies + SWDGE DMAs mid-stream; the very last
    slot uses half-granular chains with the two copies on DVE and ACT in
    parallel and SP/HWDGE DMAs to shorten the tail.
  - The host finishes with sums = acc.sum(partition), out = (O^T).T/sums.
"""

import math

import ml_dtypes
import numpy as np

import concourse.tile as tile
import concourse.mybir as mybir
from concourse import bacc
from concourse.bass_utils import run_bass_kernel_spmd

F32 = mybir.dt.float32
BF16 = mybir.dt.bfloat16

B, Q, K, D, V = 32, 1024, 1024, 128, 128
N_CORES = 8
S = B // N_CORES          # batch slots per core
CH = 128                  # K-chunk size (PE contraction width)
NCH = K // CH             # max chunks
HALF = 512                # PSUM bank limit: 512 fp32 per matmul output
SCALE = 1.0 / math.sqrt(D)
NEG_BIAS = -1.0e9


def _build(plan_key):
    """Build + compile the SPMD module.

    plan_key: per-slot (n_chunks, n_biasfree) — n_biasfree leading chunks
    are below every core's valid_len in that slot and skip the mask bias.
    """
    n_chunks = tuple(p[0] for p in plan_key)
    n_free = tuple(p[1] for p in plan_key)
    nc = bacc.Bacc("TRN2", target_bir_lowering=False, debug=False,
                   num_devices=N_CORES)
    # kt and qt packed per slot into one flat tensor: columns
    # [n_c*CH of kt | Q of qt] at offset koff[s] — one input DMA per slot.
    koff = [0]
    for s in range(S):
        koff.append(koff[-1] + n_chunks[s] * CH + Q)
    kq = nc.dram_tensor("kq", [D, koff[-1]], BF16, kind="ExternalInput")
    vt = nc.dram_tensor("vt", [S, CH, NCH, V], BF16, kind="ExternalInput")
    # Host-pre-transposed so the device DMA is a straight contiguous copy.
    mb = nc.dram_tensor("mbias", [CH, S, NCH], F32, kind="ExternalInput")
    ot = nc.dram_tensor("ot", [S, V, Q], BF16, kind="ExternalOutput")
    am = nc.dram_tensor("acc", [S, CH, Q], BF16, kind="ExternalOutput")
    # The last slot's exp chunks leave raw (host sums them): no DVE adds or
    # acc DMA on the whole-kernel tail.
    n_last = n_chunks[sorted(range(S), key=lambda i: n_chunks[i])[0]]
    el = nc.dram_tensor("el", [max(1, n_last), CH, Q], BF16,
                        kind="ExternalOutput")

    Exp = mybir.ActivationFunctionType.Exp

    # Ascending sizes with the smallest moved last: the first slot is small
    # (fast first input DMA) but, by the sorted batch assignment, still has
    # bias-free leading chunks; the smallest slot makes a short tail.
    _a = sorted(range(S), key=lambda i: n_chunks[i])
    slot_order = _a[1:] + _a[:1]
    first_s = slot_order[0]
    last_s = slot_order[-1]
    sched = [(s, c) for s in slot_order for c in range(n_chunks[s])]
    G = len(sched)

    with tile.TileContext(nc) as tc:
        with (
            tc.tile_pool(name="io", bufs=2) as io,
            tc.tile_pool(name="consts", bufs=1) as consts,
            tc.tile_pool(name="expp", bufs=5) as expp,
            tc.tile_pool(name="accp", bufs=2) as accp,
            tc.tile_pool(name="outp", bufs=2) as outp,
            tc.tile_pool(name="ps_sc", bufs=3, space="PSUM") as ps_sc_pool,
            tc.tile_pool(name="ps_ot", bufs=1, space="PSUM") as ps_ot_pool,
        ):
            # ---- lead: first slot's [kt | qt-h0] fused as the critical
            # first DMA; its qt-h1 follows as a second slice-DMA ----
            sb_kq = {}
            w0 = n_chunks[first_s] * CH + Q
            wc = w0 - HALF
            kq0 = io.tile([D, w0], BF16, tag="kq", name=f"kq{first_s}")
            nc.sync.dma_start(
                out=kq0[:, 0:wc], in_=kq.ap()[:, koff[first_s]:koff[first_s] + wc])
            nc.sync.dma_start(
                out=kq0[:, wc:w0],
                in_=kq.ap()[:, koff[first_s] + wc:koff[first_s] + w0])
            sb_kq[first_s] = kq0

            # Warm tiles via Pool (DVE is busy issuing qth1); dummy matmuls
            # keep the PE p-state ramp alive while the input DMAs land, and
            # a dummy exp pre-loads the ACT LUT table.
            warm_w = consts.tile([CH, 1], BF16)
            nc.gpsimd.memset(warm_w, 0.0)
            warm_x = consts.tile([CH, 256], BF16)
            nc.gpsimd.memset(warm_x, 0.0)
            ps_warm = ps_ot_pool.tile([1, 256], F32, tag="oth0", name="ps_warm")
            for _ in range(12):
                nc.tensor.matmul(ps_warm, lhsT=warm_w, rhs=warm_x,
                                 start=True, stop=True)
            warm_e = consts.tile([CH, 1], BF16)
            nc.scalar.activation(warm_e, warm_x[:, 0:1], func=Exp)

            # ---- remaining input DMAs, in schedule order on SP/HWDGE ----
            sb_vt = {}    # vt               [CH, n_c, V]
            vt0 = io.tile([CH, n_chunks[first_s], V], BF16, tag="vt",
                          name=f"vt{first_s}")
            nc.sync.dma_start(out=vt0,
                              in_=vt.ap()[first_s, :, 0:n_chunks[first_s], :])
            sb_vt[first_s] = vt0
            bias_t = consts.tile([CH, S, NCH], F32)
            nc.sync.dma_start(out=bias_t, in_=mb.ap())
            for s in slot_order[1:]:
                n_c = n_chunks[s]
                w = n_c * CH + Q
                kqs = io.tile([D, w], BF16, tag="kq", name=f"kq{s}")
                nc.sync.dma_start(out=kqs, in_=kq.ap()[:, koff[s]:koff[s] + w])
                sb_kq[s] = kqs
                vtt = io.tile([CH, n_c, V], BF16, tag="vt", name=f"vt{s}")
                nc.sync.dma_start(out=vtt, in_=vt.ap()[s, :, 0:n_c, :])
                sb_vt[s] = vtt

            def kt_chunk(s, c):
                return sb_kq[s][:, c * CH:(c + 1) * CH]

            def qt_half(s, h):
                base = n_chunks[s] * CH
                return sb_kq[s][:, base + h * HALF:base + (h + 1) * HALF]

            def bias_arg(s, c):
                return 0.0 if c < n_free[s] else bias_t[:, s, c:c + 1]

            # ---- flat chunk schedule, depth-2 AV software pipeline ----
            ps_ots = {}
            accs = {}
            exp_tiles = {}
            pending_add = {}

            def emit_av(g):
                s, c = sched[g]
                if c == 0:
                    if s == last_s:
                        # The scores stream is winding down: ps_sc-pool
                        # tiles free earlier than the oth ring (whose release
                        # chains through the previous slot's DVE copies).
                        # Two separate tiles so the h0/h1 output copies don't
                        # serialize on a shared-tile dependency.
                        ps_ots[s] = [
                            ps_sc_pool.tile([V, HALF], F32, tag="sc",
                                            name=f"ot_last{h}")
                            for h in range(2)
                        ]
                    else:
                        # Separate per-half O^T tiles: each output copy then
                        # waits only its own half's accumulation group.
                        ps_ots[s] = [
                            ps_ot_pool.tile([V, HALF], F32, tag=f"oth{h}",
                                            name=f"ot{s}h{h}")
                            for h in range(2)
                        ]
                e = exp_tiles.pop(g)
                vj = sb_vt[s][:, c, :]
                for h in range(2):
                    hs = slice(h * HALF, (h + 1) * HALF)
                    nc.tensor.matmul(ps_ots[s][h], lhsT=vj, rhs=e[:, hs],
                                     start=(c == 0), stop=(c == n_chunks[s] - 1))

            def finalize(s):
                """Mid-stream slot outputs: DVE copies + SWDGE DMAs. The
                slot's final acc-add was deferred here so the copies (which
                release the O^T PSUM ring) run first on the DVE; one fused
                ot DMA keeps the Pool's serial descriptor-gen off the tail."""
                sb_ot = outp.tile([V, Q], BF16, tag="otf")
                for h in range(2):
                    hs = slice(h * HALF, (h + 1) * HALF)
                    nc.vector.tensor_copy(sb_ot[:, hs], ps_ots[s][h])
                nc.gpsimd.dma_start(out=ot.ap()[s], in_=sb_ot)

            def av_and_finalize(g):
                s, c = sched[g]
                emit_av(g)
                if c == n_chunks[s] - 1 and s != last_s:
                    finalize(s)

            next_av = 0
            for g, (s, c) in enumerate(sched):
                sb_exp = expp.tile([CH, Q], BF16, tag="e")
                tail = (g == G - 1)
                hsplit = (g == 0)
                if hsplit:
                    # Two independent half-tiles from the same rotation, so
                    # the h1 score matmul doesn't falsely wait on the h0 exp
                    # reading a shared tile.
                    for h in range(2):
                        hs = slice(h * HALF, (h + 1) * HALF)
                        ps_h = ps_sc_pool.tile([CH, HALF], F32, tag="sc",
                                               name=f"sc{g}h{h}")
                        nc.tensor.matmul(ps_h, lhsT=kt_chunk(s, c),
                                         rhs=qt_half(s, h), start=True,
                                         stop=True)
                        nc.scalar.activation(
                            sb_exp[:, hs], ps_h, func=Exp,
                            bias=bias_arg(s, c), scale=SCALE)
                else:
                    ps_sc = ps_sc_pool.tile([CH, Q], F32, tag="sc")
                    for h in range(2):
                        hs = slice(h * HALF, (h + 1) * HALF)
                        nc.tensor.matmul(ps_sc[:, hs], lhsT=kt_chunk(s, c),
                                         rhs=qt_half(s, h), start=True,
                                         stop=True)
                    nc.scalar.activation(sb_exp, ps_sc, func=Exp,
                                         bias=bias_arg(s, c), scale=SCALE)
                exp_tiles[g] = sb_exp
                # Depth-2 AV pipeline mid-stream; once the last slot starts,
                # drain the previous slot fully so nothing of it trails into
                # the whole-kernel tail.
                hi = (g - 1) if s == last_s else (g - 2)
                while next_av <= hi:
                    av_and_finalize(next_av)
                    next_av += 1
                if s == last_s:
                    # Raw exp out; the host folds it into the denominator.
                    nc.sync.dma_start(out=el.ap()[c], in_=sb_exp)
                    if tail:
                        break
                    continue
                if c == 0:
                    accs[s] = accp.tile([CH, Q], BF16, tag="acc",
                                        name=f"acc{s}")
                # Denominator partials on DVE (bf16 2x mode); the slot's acc
                # leaves right after its last add, ahead of the tail's
                # DMA-queue rush.
                if c == 0:
                    nc.vector.tensor_copy(accs[s], sb_exp)
                else:
                    nc.vector.tensor_add(accs[s], accs[s], sb_exp)
                if c == n_chunks[s] - 1:
                    nc.gpsimd.dma_start(out=am.ap()[s], in_=accs[s])

            # ---- whole-kernel tail: parallel copies (DVE for h0, ACT for
            # h1) into one SBUF tile, single ot + acc DMAs on SP/HWDGE ----
            while next_av <= G - 2:
                av_and_finalize(next_av)
                next_av += 1
            s, c = sched[G - 1]
            emit_av(G - 1)
            ot_t = outp.tile([V, Q], BF16, tag="ott")
            nc.scalar.copy(ot_t[:, HALF:Q], ps_ots[s][1])
            nc.vector.tensor_copy(ot_t[:, 0:HALF], ps_ots[s][0])
            nc.sync.dma_start(out=ot.ap()[s], in_=ot_t)
    nc.compile()
    return nc


_MODULE_CACHE = {}


def _get_module(plan_key):
    key = tuple(plan_key)
    if key not in _MODULE_CACHE:
        _MODULE_CACHE[key] = _build(key)
    return _MODULE_CACHE[key]


def _plan(L):
    """Assign batches to (slot, core) sorted by valid_len.

    Returns (grid, plan_key): grid[s, c] = batch index; plan_key[s] =
    (n_chunks, n_biasfree) for slot s.
    """
    order = np.argsort(L, kind="stable")
    grid = order.reshape(S, N_CORES)       # grid[s, c] = batch index
    plan_key = []
    for s in range(S):
        mx = int(L[grid[s, -1]])
        mn = int(L[grid[s, 0]])
        n_c = max(1, (mx + CH - 1) // CH)
        plan_key.append((n_c, min(n_c, mn // CH)))
    return grid, tuple(plan_key)


def _prepare_inputs(q, k, v, L, grid, plan_key):
    kidx = np.arange(K).reshape(NCH, CH).T      # [CH, NCH] k index per (p, chunk)
    n_chunks = [p[0] for p in plan_key]
    tot = sum(n_c * CH + Q for n_c in n_chunks)
    in_maps = []
    for c in range(N_CORES):
        bs = grid[:, c]
        qt_c = q[bs].transpose(0, 2, 1)                          # [S, D, Q]
        kt_c = k[bs].transpose(0, 2, 1)                          # [S, D, K]
        kq_c = np.empty((D, tot), np.float32)
        off = 0
        for s in range(S):
            kw = n_chunks[s] * CH
            kq_c[:, off:off + kw] = kt_c[s][:, :kw]
            kq_c[:, off + kw:off + kw + Q] = qt_c[s]
            off += kw + Q
        kq_c = kq_c.astype(ml_dtypes.bfloat16)
        # [S, K, V] -> [S, CH, NCH, V]: chunk j, in-chunk row p = k index j*CH+p
        vt_c = np.ascontiguousarray(
            v[bs].reshape(S, NCH, CH, V).transpose(0, 2, 1, 3)
        ).astype(ml_dtypes.bfloat16)
        mb_c = np.empty((CH, S, NCH), np.float32)
        for s in range(S):
            mb_c[:, s] = np.where(kidx < int(L[grid[s, c]]), 0.0, NEG_BIAS)
        in_maps.append({"kq": kq_c, "vt": vt_c, "mbias": mb_c})
    return in_maps


def _postprocess(results, grid, plan_key):
    n_chunks = [p[0] for p in plan_key]
    last_s = sorted(range(S), key=lambda i: n_chunks[i])[0]
    out = np.empty((B, Q, V), np.float32)
    for c in range(N_CORES):
        otc = results[c]["ot"].astype(np.float32)                # [S, V, Q]
        sums = results[c]["acc"].astype(np.float32).sum(axis=1)  # [S, Q]
        # The last-processed slot's exp chunks left raw; sum them here.
        sums[last_s] = (
            results[c]["el"][:n_chunks[last_s]].astype(np.float32)
            .sum(axis=(0, 1))
        )
        for s in range(S):
            b = grid[s, c]
            out[b] = (otc[s] / sums[s][None, :]).T
    return out


def kernel(**inputs):
    q = np.ascontiguousarray(np.asarray(inputs["queries"], dtype=np.float32))
    k = np.ascontiguousarray(np.asarray(inputs["keys"], dtype=np.float32))
    v = np.ascontiguousarray(np.asarray(inputs["values"], dtype=np.float32))
    L = np.clip(np.asarray(inputs["valid_lens"]).astype(np.int64).reshape(-1), 1, K)
    grid, plan_key = _plan(L)
    nc = _get_module(plan_key)
    in_maps = _prepare_inputs(q, k, v, L, grid, plan_key)
    res = run_bass_kernel_spmd(nc, in_maps, core_ids=list(range(N_CORES)))
    return _postprocess(res.results, grid, plan_key)

